# revision 4
# baseline (speedup 1.0000x reference)
"""Trainium2 Bass kernel for nn_ClusteringLayer (vq_codebook).

Computes, for x (B,D) and clusters (K,D):
    sq   = ||x_i||^2 - 2 x.clusters^T + ||c_j||^2     (B,K)
    dist = sqrt(sq)
    num  = 1 / (1 + dist)          (ALPHA=1 -> exponent -1)
    out  = num / sum(num)          (global scalar normalizer)

Sharding: data-parallel on batch across 8 NeuronCores; clusters
replicated; one 4-byte AllReduce for the normalizer.

Host-side prep is layout/precision only: x and clusters are shipped as
fp8e4m3 in a DoubleRow-friendly layout ([128, 4 k-subtiles, n]) plus a
row-major fp8 copy of x for the on-device row-norm computation. Using
the SAME fp8 values for the GEMM and for x2/c2 makes dist the exact
distance between the quantized points (errors correlate and largely
cancel), ~0.1% relative error on num vs the 2e-2 gate.

Per-core device program (Bl = B/8 = 2048 local rows, 16 m-tiles):
  - warmup 4-byte AllReduce as the very first instruction (starts the
    ncfw wake/barrier immediately; result unused)
  - c2/2 row via Square + ones-matmuls, replicated across partitions
    with one K=1 fp32 matmul -> c2rep (128,1024)
  - x2 per m-tile via ACT Square with accum_out on row-major x tiles
  - per m-tile: 4 fp8 DoubleRow matmuls -> psum = x.c^T;
    GpSimd psum -= c2rep; ACT dist = Sqrt(-2*psum + x2[P,1]);
    GpSimd dist += 1; DVE reciprocal_approx_fast -> num;
    DVE tensor_reduce -> per-tile partial sums
  - cross-partition sum via one fp32 matmul; AllReduce scalar;
    reciprocal -> inv broadcast via K=1 fp32 matmul
  - scale (DVE/GpSimd alternating per 2-tile chunk) + immediate DMA out
"""

import numpy as np

B, D, K = 16384, 512, 1024
N_CORES = 8
BL = B // N_CORES        # 2048 rows per core
P = 128                  # partitions
MT = BL // P             # 16 m-tiles per core
KC = D // P              # 4 contraction subtiles
NJ = 512                 # matmul moving free dim limit (1 PSUM bank)
JH = K // NJ             # 2 j-halves

OUT_BF16 = True          # ship output as bf16, upcast on host

_CACHE = {}


def _build_bass():
    import concourse.bass as bass  # noqa: F401
    import concourse.mybir as mybir
    import concourse.tile as tile
    from concourse import bacc

    f32 = mybir.dt.float32
    bf16 = mybir.dt.bfloat16
    f8 = mybir.dt.float8e4
    AF = mybir.ActivationFunctionType
    ALU = mybir.AluOpType
    DR = mybir.MatmulPerfMode.DoubleRow
    out_dt = bf16 if OUT_BF16 else f32

    nc = bacc.Bacc(
        "TRN2", target_bir_lowering=False, debug=False, num_devices=N_CORES
    )
    xT_d = nc.dram_tensor("xT", [P, KC, BL], f8, kind="ExternalInput").ap()
    cT_d = nc.dram_tensor("cT", [P, KC, K], f8, kind="ExternalInput").ap()
    xr_d = nc.dram_tensor("xr", [BL, D], f8, kind="ExternalInput").ap()
    out_d = nc.dram_tensor("out", [BL, K], out_dt, kind="ExternalOutput").ap()

    with tile.TileContext(nc) as tc:
        with (
            tc.tile_pool(name="const", bufs=1) as cpool,
            tc.tile_pool(name="big", bufs=1) as bpool,
            tc.tile_pool(name="xrp", bufs=4) as xrp,
            tc.tile_pool(name="sq", bufs=2) as sqpool,
            tc.tile_pool(name="prow", bufs=2, space="PSUM") as prow,
            tc.tile_pool(name="pmm", bufs=3, space="PSUM") as pmm,
            tc.tile_pool(name="dram", bufs=1, space="DRAM") as dpool,
        ):
            # ---- warmup AllReduce: very first instruction, no input
            # deps, so the ncfw wake + replica barrier start at t~0.
            with tc.high_priority():
                cc_w_in = dpool.tile([1, 1], f32)
                cc_w_out = dpool.tile([1, 1], f32, addr_space="Shared")
                nc.gpsimd.collective_compute(
                    "AllReduce",
                    ALU.add,
                    replica_groups=[list(range(N_CORES))],
                    ins=[cc_w_in.opt()],
                    outs=[cc_w_out.opt()],
                )

            ones_colb = cpool.tile([P, 1], bf16)  # lhsT for c2 row sums
            nc.gpsimd.memset(ones_colb, 1.0)
            ones_col_f = cpool.tile([P, 1], f32)  # rhs for partition sum
            nc.gpsimd.memset(ones_col_f, 1.0)
            ones_row_f = cpool.tile([1, P], f32)  # lhsT for broadcasts
            nc.gpsimd.memset(ones_row_f, 1.0)

            # ---- loads (k-pair granularity so matmuls start early) ----
            ct = bpool.tile([P, KC, K], f8, name="cT")
            nc.sync.dma_start(ct[:, 0:2, :], cT_d[:, 0:2, :])
            nc.sync.dma_start(ct[:, 2:4, :], cT_d[:, 2:4, :])
            xt = bpool.tile([P, KC, BL], f8, name="xT")
            nc.sync.dma_start(xt[:, 0:2, :], xT_d[:, 0:2, :])
            nc.sync.dma_start(xt[:, 2:4, :], xT_d[:, 2:4, :])
            xrts = []
            for i in range(MT):
                xrt = xrp.tile([P, D], f8, tag="xr")
                nc.sync.dma_start(xrt, xr_d[i * P : (i + 1) * P, :])
                xrts.append(xrt)

            # ---- c2/2 replicated row (one-time) ----
            csqs = []
            for k in range(KC):
                csq = sqpool.tile([P, K], bf16, tag="csq")
                nc.vector.tensor_mul(csq, ct[:, k, :], ct[:, k, :])
                csqs.append(csq)
            c2h = cpool.tile([1, K], f32)  # c2/2 row
            for h in range(JH):
                rp = prow.tile([1, NJ], f32, tag="row")
                for k in range(KC):
                    nc.tensor.matmul(
                        rp,
                        lhsT=ones_colb,
                        rhs=csqs[k][:, h * NJ : (h + 1) * NJ],
                        start=(k == 0),
                        stop=(k == KC - 1),
                    )
                nc.scalar.activation(
                    c2h[0:1, h * NJ : (h + 1) * NJ], rp, AF.Copy, scale=0.5
                )
            c2rep = bpool.tile([P, K], f32, name="c2rep")
            pb = pmm.tile([P, K], f32, tag="mm")
            for h in range(JH):
                nc.tensor.matmul(
                    pb[:, h * NJ : (h + 1) * NJ],
                    lhsT=ones_row_f,
                    rhs=c2h[0:1, h * NJ : (h + 1) * NJ],
                    start=True,
                    stop=True,
                )
            nc.vector.tensor_copy(c2rep, pb)

            # ---- x2 per m-tile: ACT Square with accum_out ----
            x2col = cpool.tile([P, MT], f32)
            sq_scr = cpool.tile([P, D], bf16)  # write-only scratch
            for i in range(MT):
                nc.scalar.activation(
                    sq_scr, xrts[i], AF.Square,
                    accum_out=x2col[:, i : i + 1],
                )

            numbuf = bpool.tile([P, MT * K], f32)  # 64 KB/partition
            parts = cpool.tile([P, MT], f32)

            # ---- main pipeline ----
            for i in range(MT):
                ps = pmm.tile([P, K], f32, tag="mm")
                for h in range(JH):
                    psl = ps[:, h * NJ : (h + 1) * NJ]
                    for kp in range(KC // 2):
                        nc.tensor.matmul(
                            psl,
                            lhsT=xt[:, 2 * kp : 2 * kp + 2, i * P : (i + 1) * P],
                            rhs=ct[:, 2 * kp : 2 * kp + 2, h * NJ : (h + 1) * NJ],
                            start=(kp == 0),
                            stop=(kp == KC // 2 - 1),
                            perf_mode=DR,
                        )
                nsl = numbuf[:, i * K : (i + 1) * K]
                # psum = x.c - c2/2 ; dist = sqrt(-2*psum + x2)
                # (GpSimd cannot access PSUM on TRN2 -> DVE)
                nc.vector.tensor_tensor(
                    out=ps, in0=ps, in1=c2rep, op=ALU.subtract
                )
                nc.scalar.activation(
                    nsl, ps, AF.Sqrt,
                    bias=x2col[:, i : i + 1], scale=-2.0,
                )
                nc.gpsimd.tensor_scalar_add(nsl, nsl, 1.0)
                nc.vector.reciprocal_approx_fast(nsl, nsl)
                nc.vector.tensor_reduce(
                    parts[:, i : i + 1], nsl, mybir.AxisListType.X, ALU.add
                )

            # ---- local total: [P,MT] -> [P,1] -> [1,1] ----
            pacc = cpool.tile([P, 1], f32)
            nc.vector.tensor_reduce(pacc, parts, mybir.AxisListType.X, ALU.add)
            sp = prow.tile([1, 1], f32, tag="row")
            nc.tensor.matmul(sp, lhsT=pacc, rhs=ones_col_f, start=True, stop=True)
            lsum = cpool.tile([1, 1], f32)
            nc.vector.tensor_copy(lsum, sp)

            # ---- AllReduce the scalar, then inv broadcast ----
            cc_in = dpool.tile([1, 1], f32)
            cc_out = dpool.tile([1, 1], f32, addr_space="Shared")
            nc.sync.dma_start(cc_in, lsum)
            nc.gpsimd.collective_compute(
                "AllReduce",
                ALU.add,
                replica_groups=[list(range(N_CORES))],
                ins=[cc_in.opt()],
                outs=[cc_out.opt()],
            )
            total = cpool.tile([1, 1], f32)
            nc.sync.dma_start(total, cc_out)
            inv = cpool.tile([1, 1], f32)
            nc.vector.reciprocal(inv, total)
            inv_ps = prow.tile([P, 1], f32, tag="row")
            nc.tensor.matmul(inv_ps, lhsT=ones_row_f, rhs=inv, start=True, stop=True)
            invb = cpool.tile([P, 1], f32)
            nc.vector.tensor_copy(invb, inv_ps)

            # ---- scale (DVE/GpSimd alternating) + store per 2 tiles ----
            if OUT_BF16:
                obuf = bpool.tile([P, MT * K], out_dt, name="obuf")
            else:
                obuf = numbuf
            NM = 2  # m-tiles per chunk
            for s in range(MT // NM):
                i0 = s * NM
                sl = numbuf[:, i0 * K : (i0 + NM) * K]
                osl = obuf[:, i0 * K : (i0 + NM) * K]
                eng = nc.vector if s % 2 == 0 else nc.gpsimd
                eng.tensor_scalar_mul(osl, sl, invb)
                dst = out_d[i0 * P : (i0 + NM) * P, :].rearrange(
                    "(f p) c -> p f c", p=P
                )
                srcb = osl.rearrange("p (f c) -> p f c", f=NM)
                nc.sync.dma_start(dst, srcb)

    nc.finalize()
    return nc


def _get_bass():
    key = "nc"
    if key not in _CACHE:
        _CACHE[key] = _build_bass()
    return _CACHE[key]


def _host_prep(x: np.ndarray, clusters: np.ndarray):
    import ml_dtypes

    f8 = ml_dtypes.float8_e4m3
    # [128, KC, n] DoubleRow layout: tile[p, j, n] = src[n, j*128+p]
    cT = np.ascontiguousarray(
        clusters.T.reshape(KC, P, K).transpose(1, 0, 2)
    ).astype(f8)
    in_maps = []
    for c in range(N_CORES):
        xl = x[c * BL : (c + 1) * BL]
        xT_c = np.ascontiguousarray(
            xl.T.reshape(KC, P, BL).transpose(1, 0, 2)
        ).astype(f8)
        xr_c = np.ascontiguousarray(xl).astype(f8)
        in_maps.append({"xT": xT_c, "cT": cT, "xr": xr_c})
    return in_maps


def kernel(x: np.ndarray, clusters: np.ndarray) -> np.ndarray:
    from concourse.bass_utils import run_bass_kernel_spmd

    x = np.asarray(x, dtype=np.float32)
    clusters = np.asarray(clusters, dtype=np.float32)
    assert x.shape == (B, D) and clusters.shape == (K, D)

    in_maps = _host_prep(x, clusters)
    nc = _get_bass()
    res = run_bass_kernel_spmd(nc, in_maps, core_ids=list(range(N_CORES)))
    return np.concatenate(
        [np.asarray(r["out"]).astype(np.float32) for r in res.results], axis=0
    )


# revision 9
# speedup vs baseline: 2.9504x; 2.9504x over previous
"""Trainium2 Bass kernel for nn_ClusteringLayer (vq_codebook).

Computes, for x (B,D) and clusters (K,D):
    sq   = ||x_i||^2 - 2 x.clusters^T + ||c_j||^2     (B,K)
    dist = sqrt(sq)
    num  = 1 / (1 + dist)          (ALPHA=1 -> exponent -1)
    out  = num / sum(num)          (global scalar normalizer)

Sharding: data-parallel on batch across 8 NeuronCores; clusters
replicated; one 4-byte AllReduce for the normalizer.

Host-side prep is layout/precision only: x and clusters are shipped as
fp8e4m3 in a DoubleRow-friendly layout ([128, 4 k-subtiles, n]) plus a
row-major fp8 copy of x for the on-device row-norm computation. Using
the SAME fp8 values for the GEMM and for x2/c2 makes dist the exact
distance between the quantized points (errors correlate and largely
cancel), ~0.1% relative error on num vs the 2e-2 gate.

Per-core device program (Bl = B/8 = 2048 local rows, 16 m-tiles):
  - warmup 4-byte AllReduce as the very first instruction (starts the
    ncfw wake/barrier immediately; result unused)
  - c2/2 row via Square + ones-matmuls, replicated across partitions
    with one K=1 fp32 matmul -> c2rep (128,1024)
  - x2 per m-tile via ACT Square with accum_out on row-major x tiles
  - per m-tile: 4 fp8 DoubleRow matmuls -> psum = x.c^T;
    GpSimd psum -= c2rep; ACT dist = Sqrt(-2*psum + x2[P,1]);
    GpSimd dist += 1; DVE reciprocal_approx_fast -> num;
    DVE tensor_reduce -> per-tile partial sums
  - cross-partition sum via one fp32 matmul; AllReduce scalar;
    reciprocal -> inv broadcast via K=1 fp32 matmul
  - scale (DVE/GpSimd alternating per 2-tile chunk) + immediate DMA out
"""

import numpy as np

B, D, K = 16384, 512, 1024
N_CORES = 8
BL = B // N_CORES        # 2048 rows per core
P = 128                  # partitions
MT = BL // P             # 16 m-tiles per core
KC = D // P              # 4 contraction subtiles
NJ = 512                 # matmul moving free dim limit (1 PSUM bank)
JH = K // NJ             # 2 j-halves

OUT_BF16 = False         # ship output as bf16, upcast on host

_CACHE = {}


def _build_bass():
    import concourse.bass as bass  # noqa: F401
    import concourse.mybir as mybir
    import concourse.tile as tile
    from concourse import bacc

    f32 = mybir.dt.float32
    bf16 = mybir.dt.bfloat16
    f8 = mybir.dt.float8e4
    AF = mybir.ActivationFunctionType
    ALU = mybir.AluOpType
    DR = mybir.MatmulPerfMode.DoubleRow
    out_dt = bf16 if OUT_BF16 else f32

    nc = bacc.Bacc(
        "TRN2", target_bir_lowering=False, debug=False, num_devices=N_CORES
    )
    xT_d = nc.dram_tensor("xT", [P, KC, BL], f8, kind="ExternalInput").ap()
    cT_d = nc.dram_tensor("cT", [P, KC, K], f8, kind="ExternalInput").ap()
    xr_d = nc.dram_tensor("xr", [BL, D], f8, kind="ExternalInput").ap()
    out_d = nc.dram_tensor("out", [BL, K], out_dt, kind="ExternalOutput").ap()

    with tile.TileContext(nc) as tc:
        with (
            tc.tile_pool(name="const", bufs=1) as cpool,
            tc.tile_pool(name="big", bufs=1) as bpool,
            tc.tile_pool(name="xrp", bufs=4) as xrp,
            tc.tile_pool(name="sq", bufs=2) as sqpool,
            tc.tile_pool(name="ln", bufs=2) as lpool,
            tc.tile_pool(name="prow", bufs=2, space="PSUM") as prow,
            tc.tile_pool(name="pmm", bufs=3, space="PSUM") as pmm,
            tc.tile_pool(name="dram", bufs=1, space="DRAM") as dpool,
        ):
            # ---- warmup AllReduce: very first instruction, no input
            # deps, so the ncfw wake + replica barrier start at t~0.
            with tc.high_priority():
                cc_w_in = dpool.tile([1, 1], f32)
                cc_w_out = dpool.tile([1, 1], f32, addr_space="Shared")
                nc.gpsimd.collective_compute(
                    "AllReduce",
                    ALU.add,
                    replica_groups=[list(range(N_CORES))],
                    ins=[cc_w_in.opt()],
                    outs=[cc_w_out.opt()],
                )

            ones_colb = cpool.tile([P, 1], bf16)  # lhsT for c2 row sums
            nc.vector.memset(ones_colb, 1.0)
            ones_col_f = cpool.tile([P, 1], f32)  # rhs for partition sum
            nc.vector.memset(ones_col_f, 1.0)
            ones_row_f = cpool.tile([1, P], f32)  # lhsT for broadcasts
            nc.vector.memset(ones_row_f, 1.0)

            # ---- loads (k-pair granularity so matmuls start early) ----
            ct = bpool.tile([P, KC, K], f8, name="cT")
            nc.sync.dma_start(ct[:, 0:2, :], cT_d[:, 0:2, :])
            nc.sync.dma_start(ct[:, 2:4, :], cT_d[:, 2:4, :])
            xt = bpool.tile([P, KC, BL], f8, name="xT")
            nc.sync.dma_start(xt[:, 0:2, :], xT_d[:, 0:2, :])
            nc.sync.dma_start(xt[:, 2:4, :], xT_d[:, 2:4, :])
            xrts = []
            for i in range(MT):
                xrt = xrp.tile([P, D], f8, tag="xr")
                nc.sync.dma_start(xrt, xr_d[i * P : (i + 1) * P, :])
                xrts.append(xrt)

            # ---- c2/2 replicated row (one-time) ----
            csqs = []
            for k in range(KC):
                csq = sqpool.tile([P, K], bf16, tag="csq")
                nc.vector.tensor_mul(csq, ct[:, k, :], ct[:, k, :])
                csqs.append(csq)
            c2h = cpool.tile([1, K], f32)  # c2/2 row
            for h in range(JH):
                rp = prow.tile([1, NJ], f32, tag="row")
                for k in range(KC):
                    nc.tensor.matmul(
                        rp,
                        lhsT=ones_colb,
                        rhs=csqs[k][:, h * NJ : (h + 1) * NJ],
                        start=(k == 0),
                        stop=(k == KC - 1),
                    )
                nc.scalar.activation(
                    c2h[0:1, h * NJ : (h + 1) * NJ], rp, AF.Copy, scale=0.5
                )
            c2rep = bpool.tile([P, K], f32, name="c2rep")
            pb = pmm.tile([P, K], f32, tag="mm")
            for h in range(JH):
                nc.tensor.matmul(
                    pb[:, h * NJ : (h + 1) * NJ],
                    lhsT=ones_row_f,
                    rhs=c2h[0:1, h * NJ : (h + 1) * NJ],
                    start=True,
                    stop=True,
                )
            nc.vector.tensor_copy(c2rep, pb)

            # ---- x2 per m-tile: ACT Square with accum_out ----
            x2col = cpool.tile([P, MT], f32)
            sq_scr = cpool.tile([P, D], bf16)  # write-only scratch
            for i in range(MT):
                nc.scalar.activation(
                    sq_scr, xrts[i], AF.Square,
                    accum_out=x2col[:, i : i + 1],
                )

            numbuf = bpool.tile([P, MT * K], f32)  # 64 KB/partition
            parts = cpool.tile([P, MT], f32)

            # ---- main pipeline ----
            # num = 1/(1+dist) = Sigmoid(-0.5*Ln(sq)): two ACT table
            # passes, no sqrt/add/reciprocal, and the Sigmoid pass's
            # accum_out produces the per-row partial sums for free.
            for i in range(MT):
                ps = pmm.tile([P, K], f32, tag="mm")
                for h in range(JH):
                    psl = ps[:, h * NJ : (h + 1) * NJ]
                    for kp in range(KC // 2):
                        nc.tensor.matmul(
                            psl,
                            lhsT=xt[:, 2 * kp : 2 * kp + 2, i * P : (i + 1) * P],
                            rhs=ct[:, 2 * kp : 2 * kp + 2, h * NJ : (h + 1) * NJ],
                            start=(kp == 0),
                            stop=(kp == KC // 2 - 1),
                            perf_mode=DR,
                        )
                nsl = numbuf[:, i * K : (i + 1) * K]
                # psum = x.c - c2/2 ; sq = -2*psum + x2
                # (GpSimd cannot access PSUM on TRN2 -> DVE)
                nc.vector.tensor_tensor(
                    out=ps, in0=ps, in1=c2rep, op=ALU.subtract
                )
                lsc = lpool.tile([P, K], f32, tag="ln")
                nc.scalar.activation(
                    lsc, ps, AF.Ln,
                    bias=x2col[:, i : i + 1], scale=-2.0,
                )
                nc.scalar.activation(
                    nsl, lsc, AF.Sigmoid, scale=-0.5,
                    accum_out=parts[:, i : i + 1],
                )

            # ---- local total: [P,MT] -> [P,1] -> [1,1] ----
            pacc = cpool.tile([P, 1], f32)
            nc.vector.tensor_reduce(pacc, parts, mybir.AxisListType.X, ALU.add)
            sp = prow.tile([1, 1], f32, tag="row")
            nc.tensor.matmul(sp, lhsT=pacc, rhs=ones_col_f, start=True, stop=True)
            lsum = cpool.tile([1, 1], f32)
            nc.vector.tensor_copy(lsum, sp)

            # ---- AllReduce the scalar, then inv broadcast ----
            cc_in = dpool.tile([1, 1], f32)
            cc_out = dpool.tile([1, 1], f32, addr_space="Shared")
            nc.sync.dma_start(cc_in, lsum)
            nc.gpsimd.collective_compute(
                "AllReduce",
                ALU.add,
                replica_groups=[list(range(N_CORES))],
                ins=[cc_in.opt()],
                outs=[cc_out.opt()],
            )
            total = cpool.tile([1, 1], f32)
            nc.sync.dma_start(total, cc_out)
            inv = cpool.tile([1, 1], f32)
            nc.vector.reciprocal(inv, total)
            inv_ps = prow.tile([P, 1], f32, tag="row")
            nc.tensor.matmul(inv_ps, lhsT=ones_row_f, rhs=inv, start=True, stop=True)
            invb = cpool.tile([P, 1], f32)
            nc.vector.tensor_copy(invb, inv_ps)

            # ---- scale (DVE, fp32 fast path) + store per 2 tiles ----
            NM = 2  # m-tiles per chunk
            for s in range(MT // NM):
                i0 = s * NM
                sl = numbuf[:, i0 * K : (i0 + NM) * K]
                nc.vector.tensor_scalar_mul(sl, sl, invb)
                dst = out_d[i0 * P : (i0 + NM) * P, :].rearrange(
                    "(f p) c -> p f c", p=P
                )
                srcb = sl.rearrange("p (f c) -> p f c", f=NM)
                nc.sync.dma_start(dst, srcb)

    nc.finalize()
    return nc


def _get_bass():
    key = "nc"
    if key not in _CACHE:
        _CACHE[key] = _build_bass()
    return _CACHE[key]


def _host_prep(x: np.ndarray, clusters: np.ndarray):
    import ml_dtypes

    f8 = ml_dtypes.float8_e4m3
    # [128, KC, n] DoubleRow layout: tile[p, j, n] = src[n, j*128+p]
    cT = np.ascontiguousarray(
        clusters.T.reshape(KC, P, K).transpose(1, 0, 2)
    ).astype(f8)
    in_maps = []
    for c in range(N_CORES):
        xl = x[c * BL : (c + 1) * BL]
        xT_c = np.ascontiguousarray(
            xl.T.reshape(KC, P, BL).transpose(1, 0, 2)
        ).astype(f8)
        xr_c = np.ascontiguousarray(xl).astype(f8)
        in_maps.append({"xT": xT_c, "cT": cT, "xr": xr_c})
    return in_maps


def kernel(x: np.ndarray, clusters: np.ndarray) -> np.ndarray:
    from concourse.bass_utils import run_bass_kernel_spmd

    x = np.asarray(x, dtype=np.float32)
    clusters = np.asarray(clusters, dtype=np.float32)
    assert x.shape == (B, D) and clusters.shape == (K, D)

    in_maps = _host_prep(x, clusters)
    nc = _get_bass()
    res = run_bass_kernel_spmd(nc, in_maps, core_ids=list(range(N_CORES)))
    return np.concatenate(
        [np.asarray(r["out"]).astype(np.float32) for r in res.results], axis=0
    )


# revision 14
# speedup vs baseline: 3.2155x; 1.0898x over previous
"""Trainium2 Bass kernel for nn_ClusteringLayer (vq_codebook).

Computes, for x (B,D) and clusters (K,D):
    sq   = ||x_i||^2 - 2 x.clusters^T + ||c_j||^2     (B,K)
    dist = sqrt(sq)
    num  = 1 / (1 + dist)          (ALPHA=1 -> exponent -1)
    out  = num / sum(num)          (global scalar normalizer)

Sharding: data-parallel on batch across 8 NeuronCores; clusters
replicated; one 4-byte AllReduce for the normalizer.

Host-side prep is layout/precision only: x and clusters are shipped as
fp8e4m3 in a DoubleRow-friendly layout ([128, 4 k-subtiles, n]) plus a
row-major fp8 copy of x for the on-device row-norm computation. Using
the SAME fp8 values for the GEMM and for x2/c2 makes dist the exact
distance between the quantized points (errors correlate and largely
cancel), ~0.1% relative error on num vs the 2e-2 gate.

Per-core device program (Bl = B/8 = 2048 local rows, 16 m-tiles):
  - warmup 4-byte AllReduce as the very first instruction (starts the
    ncfw wake/barrier immediately; result unused)
  - c2/2 row via Square + ones-matmuls, replicated across partitions
    with one K=1 fp32 matmul -> c2rep (128,1024)
  - x2 per m-tile via ACT Square with accum_out on row-major x tiles
  - per m-tile: 4 fp8 DoubleRow matmuls -> psum = x.c^T;
    GpSimd psum -= c2rep; ACT dist = Sqrt(-2*psum + x2[P,1]);
    GpSimd dist += 1; DVE reciprocal_approx_fast -> num;
    DVE tensor_reduce -> per-tile partial sums
  - cross-partition sum via one fp32 matmul; AllReduce scalar;
    reciprocal -> inv broadcast via K=1 fp32 matmul
  - scale (DVE/GpSimd alternating per 2-tile chunk) + immediate DMA out
"""

import numpy as np

B, D, K = 16384, 512, 1024
N_CORES = 8
BL = B // N_CORES        # 2048 rows per core
P = 128                  # partitions
MT = BL // P             # 16 m-tiles per core
KC = D // P              # 4 contraction subtiles
NJ = 512                 # matmul moving free dim limit (1 PSUM bank)
JH = K // NJ             # 2 j-halves

OUT_BF16 = False         # ship output as bf16, upcast on host

_CACHE = {}


def _build_bass():
    import concourse.bass as bass  # noqa: F401
    import concourse.mybir as mybir
    import concourse.tile as tile
    from concourse import bacc

    f32 = mybir.dt.float32
    bf16 = mybir.dt.bfloat16
    f8 = mybir.dt.float8e4
    AF = mybir.ActivationFunctionType
    ALU = mybir.AluOpType
    DR = mybir.MatmulPerfMode.DoubleRow
    out_dt = bf16 if OUT_BF16 else f32

    nc = bacc.Bacc(
        "TRN2", target_bir_lowering=False, debug=False, num_devices=N_CORES
    )
    xT_d = nc.dram_tensor("xT", [P, KC, BL], f8, kind="ExternalInput").ap()
    cT_d = nc.dram_tensor("cT", [P, KC, K], f8, kind="ExternalInput").ap()
    xr_d = nc.dram_tensor("xr", [BL, D], f8, kind="ExternalInput").ap()
    out_d = nc.dram_tensor("out", [BL, K], out_dt, kind="ExternalOutput").ap()

    with tile.TileContext(nc) as tc:
        with (
            tc.tile_pool(name="const", bufs=1) as cpool,
            tc.tile_pool(name="big", bufs=1) as bpool,
            tc.tile_pool(name="xrp", bufs=4) as xrp,
            tc.tile_pool(name="sq", bufs=2) as sqpool,
            tc.tile_pool(name="ln", bufs=4) as lpool,
            tc.tile_pool(name="prow", bufs=2, space="PSUM") as prow,
            tc.tile_pool(name="pmm", bufs=3, space="PSUM") as pmm,
            tc.tile_pool(name="dram", bufs=1, space="DRAM") as dpool,
        ):
            # ---- warmup AllReduce: very first instruction, no input
            # deps, so the ncfw wake + replica barrier start at t~0.
            with tc.high_priority():
                cc_w_in = dpool.tile([1, 1], f32)
                cc_w_out = dpool.tile([1, 1], f32, addr_space="Shared")
                nc.gpsimd.collective_compute(
                    "AllReduce",
                    ALU.add,
                    replica_groups=[list(range(N_CORES))],
                    ins=[cc_w_in.opt()],
                    outs=[cc_w_out.opt()],
                )

            ones_colb = cpool.tile([P, 1], bf16)  # lhsT for c2 row sums
            nc.vector.memset(ones_colb, 1.0)
            ones_col_f = cpool.tile([P, 1], f32)  # rhs for partition sum
            nc.vector.memset(ones_col_f, 1.0)
            ones_row_f = cpool.tile([1, P], f32)  # lhsT for broadcasts
            nc.vector.memset(ones_row_f, 1.0)

            # ---- loads (k-pair granularity so matmuls start early) ----
            ct = bpool.tile([P, KC, K], f8, name="cT")
            nc.sync.dma_start(ct[:, 0:2, :], cT_d[:, 0:2, :])
            nc.sync.dma_start(ct[:, 2:4, :], cT_d[:, 2:4, :])
            xt = bpool.tile([P, KC, BL], f8, name="xT")
            nc.sync.dma_start(xt[:, 0:2, :], xT_d[:, 0:2, :])
            nc.sync.dma_start(xt[:, 2:4, :], xT_d[:, 2:4, :])
            xrts = []
            for i in range(MT):
                xrt = xrp.tile([P, D], f8, tag="xr")
                nc.sync.dma_start(xrt, xr_d[i * P : (i + 1) * P, :])
                xrts.append(xrt)

            # ---- c2/2 replicated row (one-time) ----
            csqs = []
            for k in range(KC):
                csq = sqpool.tile([P, K], bf16, tag="csq")
                nc.vector.tensor_mul(csq, ct[:, k, :], ct[:, k, :])
                csqs.append(csq)
            c2h = cpool.tile([1, K], f32)  # c2 row
            for h in range(JH):
                rp = prow.tile([1, NJ], f32, tag="row")
                for k in range(KC):
                    nc.tensor.matmul(
                        rp,
                        lhsT=ones_colb,
                        rhs=csqs[k][:, h * NJ : (h + 1) * NJ],
                        start=(k == 0),
                        stop=(k == KC - 1),
                    )
                nc.scalar.activation(
                    c2h[0:1, h * NJ : (h + 1) * NJ], rp, AF.Copy, scale=1.0
                )
            c2rep = bpool.tile([P, K], f32, name="c2rep")
            pb = pmm.tile([P, K], f32, tag="mm")
            for h in range(JH):
                nc.tensor.matmul(
                    pb[:, h * NJ : (h + 1) * NJ],
                    lhsT=ones_row_f,
                    rhs=c2h[0:1, h * NJ : (h + 1) * NJ],
                    start=True,
                    stop=True,
                )
            nc.vector.tensor_copy(c2rep, pb)

            x2col = cpool.tile([P, MT], f32)
            sq_scr = cpool.tile([P, D], bf16)  # write-only scratch
            numbuf = bpool.tile([P, MT * K], f32)  # 64 KB/partition
            parts = cpool.tile([P, MT], f32)

            # ---- main pipeline ----
            # num = 1/(1+dist) = Sigmoid(-0.5*Ln(sq)): two ACT table
            # passes, no sqrt/add/reciprocal, and the Sigmoid pass's
            # accum_out produces the per-row partial sums for free.
            # DVE stages t = 2*psum - c2 into numbuf (frees PSUM), and
            # Ln reads it with scale=-1, bias=x2 -> ln(x2 - 2xc + c2).
            # Ln/Sigmoid run in batches of ACTB tiles so the ACT engine
            # amortizes its ~1.3us per-function table reloads.
            ACTB = 4
            for i in range(MT):
                ps = pmm.tile([P, K], f32, tag="mm")
                for h in range(JH):
                    psl = ps[:, h * NJ : (h + 1) * NJ]
                    for kp in range(KC // 2):
                        nc.tensor.matmul(
                            psl,
                            lhsT=xt[:, 2 * kp : 2 * kp + 2, i * P : (i + 1) * P],
                            rhs=ct[:, 2 * kp : 2 * kp + 2, h * NJ : (h + 1) * NJ],
                            start=(kp == 0),
                            stop=(kp == KC // 2 - 1),
                            perf_mode=DR,
                        )
                # x2 on DVE: square + row-reduce the row-major x tile
                nc.vector.tensor_mul(sq_scr, xrts[i], xrts[i])
                nc.vector.tensor_reduce(
                    x2col[:, i : i + 1], sq_scr, mybir.AxisListType.X, ALU.add
                )
                nsl = numbuf[:, i * K : (i + 1) * K]
                nc.vector.scalar_tensor_tensor(
                    out=nsl, in0=ps, scalar=2.0, in1=c2rep,
                    op0=ALU.mult, op1=ALU.subtract,
                )
                if i % ACTB == ACTB - 1:
                    lscs = {}
                    for j in range(i - ACTB + 1, i + 1):
                        lscs[j] = lpool.tile([P, K], f32, tag="ln", name=f"lsc{j}")
                        nc.scalar.activation(
                            lscs[j], numbuf[:, j * K : (j + 1) * K], AF.Ln,
                            bias=x2col[:, j : j + 1], scale=-1.0,
                        )
                    for j in range(i - ACTB + 1, i + 1):
                        nc.scalar.activation(
                            numbuf[:, j * K : (j + 1) * K], lscs[j], AF.Sigmoid,
                            scale=-0.5, accum_out=parts[:, j : j + 1],
                        )

            # ---- local total: [P,MT] -> [P,1] -> [1,1] ----
            pacc = cpool.tile([P, 1], f32)
            nc.vector.tensor_reduce(pacc, parts, mybir.AxisListType.X, ALU.add)
            sp = prow.tile([1, 1], f32, tag="row")
            nc.tensor.matmul(sp, lhsT=pacc, rhs=ones_col_f, start=True, stop=True)
            lsum = cpool.tile([1, 1], f32)
            nc.vector.tensor_copy(lsum, sp)

            # ---- AllReduce the scalar, then inv broadcast ----
            cc_in = dpool.tile([1, 1], f32)
            cc_out = dpool.tile([1, 1], f32, addr_space="Shared")
            nc.sync.dma_start(cc_in, lsum)
            nc.gpsimd.collective_compute(
                "AllReduce",
                ALU.add,
                replica_groups=[list(range(N_CORES))],
                ins=[cc_in.opt()],
                outs=[cc_out.opt()],
            )
            total = cpool.tile([1, 1], f32)
            nc.sync.dma_start(total, cc_out)
            inv = cpool.tile([1, 1], f32)
            nc.vector.reciprocal(inv, total)
            inv_ps = prow.tile([P, 1], f32, tag="row")
            nc.tensor.matmul(inv_ps, lhsT=ones_row_f, rhs=inv, start=True, stop=True)
            invb = cpool.tile([P, 1], f32)
            nc.vector.tensor_copy(invb, inv_ps)

            # ---- scale (DVE, fp32 fast path) + store per 2 tiles ----
            NM = 2  # m-tiles per chunk
            for s in range(MT // NM):
                i0 = s * NM
                sl = numbuf[:, i0 * K : (i0 + NM) * K]
                nc.vector.tensor_scalar_mul(sl, sl, invb)
                dst = out_d[i0 * P : (i0 + NM) * P, :].rearrange(
                    "(f p) c -> p f c", p=P
                )
                srcb = sl.rearrange("p (f c) -> p f c", f=NM)
                nc.sync.dma_start(dst, srcb)

    nc.finalize()
    return nc


def _get_bass():
    key = "nc"
    if key not in _CACHE:
        _CACHE[key] = _build_bass()
    return _CACHE[key]


def _host_prep(x: np.ndarray, clusters: np.ndarray):
    import ml_dtypes

    f8 = ml_dtypes.float8_e4m3
    # [128, KC, n] DoubleRow layout: tile[p, j, n] = src[n, j*128+p]
    cT = np.ascontiguousarray(
        clusters.T.reshape(KC, P, K).transpose(1, 0, 2)
    ).astype(f8)
    in_maps = []
    for c in range(N_CORES):
        xl = x[c * BL : (c + 1) * BL]
        xT_c = np.ascontiguousarray(
            xl.T.reshape(KC, P, BL).transpose(1, 0, 2)
        ).astype(f8)
        xr_c = np.ascontiguousarray(xl).astype(f8)
        in_maps.append({"xT": xT_c, "cT": cT, "xr": xr_c})
    return in_maps


def kernel(x: np.ndarray, clusters: np.ndarray) -> np.ndarray:
    from concourse.bass_utils import run_bass_kernel_spmd

    x = np.asarray(x, dtype=np.float32)
    clusters = np.asarray(clusters, dtype=np.float32)
    assert x.shape == (B, D) and clusters.shape == (K, D)

    in_maps = _host_prep(x, clusters)
    nc = _get_bass()
    res = run_bass_kernel_spmd(nc, in_maps, core_ids=list(range(N_CORES)))
    return np.concatenate(
        [np.asarray(r["out"]).astype(np.float32) for r in res.results], axis=0
    )


# revision 19
# speedup vs baseline: 3.3965x; 1.0563x over previous
"""Trainium2 Bass kernel for nn_ClusteringLayer (vq_codebook).

Computes, for x (B,D) and clusters (K,D):
    sq   = ||x_i||^2 - 2 x.clusters^T + ||c_j||^2     (B,K)
    dist = sqrt(sq)
    num  = 1 / (1 + dist)          (ALPHA=1 -> exponent -1)
    out  = num / sum(num)          (global scalar normalizer)

Sharding: data-parallel on batch across 8 NeuronCores; clusters
replicated; one 4-byte AllReduce for the normalizer.

Host-side prep is layout/precision only: x and clusters are shipped as
fp8e4m3 in a DoubleRow-friendly layout ([128, 4 k-subtiles, n]) plus a
row-major fp8 copy of x for the on-device row-norm computation. Using
the SAME fp8 values for the GEMM and for x2/c2 makes dist the exact
distance between the quantized points (errors correlate and largely
cancel), ~0.1% relative error on num vs the 2e-2 gate.

Per-core device program (Bl = B/8 = 2048 local rows, 16 m-tiles):
  - warmup 4-byte AllReduce as the very first instruction (starts the
    ncfw wake/barrier immediately; result unused)
  - c2/2 row via Square + ones-matmuls, replicated across partitions
    with one K=1 fp32 matmul -> c2rep (128,1024)
  - x2 per m-tile via ACT Square with accum_out on row-major x tiles
  - per m-tile: 4 fp8 DoubleRow matmuls -> psum = x.c^T;
    GpSimd psum -= c2rep; ACT dist = Sqrt(-2*psum + x2[P,1]);
    GpSimd dist += 1; DVE reciprocal_approx_fast -> num;
    DVE tensor_reduce -> per-tile partial sums
  - cross-partition sum via one fp32 matmul; AllReduce scalar;
    reciprocal -> inv broadcast via K=1 fp32 matmul
  - scale (DVE/GpSimd alternating per 2-tile chunk) + immediate DMA out
"""

import numpy as np

B, D, K = 16384, 512, 1024
N_CORES = 8
BL = B // N_CORES        # 2048 rows per core
P = 128                  # partitions
MT = BL // P             # 16 m-tiles per core
KC = D // P              # 4 contraction subtiles
NJ = 512                 # matmul moving free dim limit (1 PSUM bank)
JH = K // NJ             # 2 j-halves

OUT_BF16 = True          # ship output as bf16, upcast on host

_CACHE = {}


def _build_bass():
    import concourse.bass as bass  # noqa: F401
    import concourse.mybir as mybir
    import concourse.tile as tile
    from concourse import bacc

    f32 = mybir.dt.float32
    bf16 = mybir.dt.bfloat16
    f8 = mybir.dt.float8e4
    AF = mybir.ActivationFunctionType
    ALU = mybir.AluOpType
    DR = mybir.MatmulPerfMode.DoubleRow
    out_dt = bf16 if OUT_BF16 else f32

    nc = bacc.Bacc(
        "TRN2", target_bir_lowering=False, debug=False, num_devices=N_CORES
    )
    xT_d = nc.dram_tensor("xT", [P, KC, BL], f8, kind="ExternalInput").ap()
    cT_d = nc.dram_tensor("cT", [P, KC, K], f8, kind="ExternalInput").ap()
    xr_d = nc.dram_tensor("xr", [BL, D], f8, kind="ExternalInput").ap()
    out_d = nc.dram_tensor("out", [BL, K], out_dt, kind="ExternalOutput").ap()

    with tile.TileContext(nc) as tc:
        with (
            tc.tile_pool(name="const", bufs=1) as cpool,
            tc.tile_pool(name="big", bufs=1) as bpool,
            tc.tile_pool(name="xrp", bufs=4) as xrp,
            tc.tile_pool(name="sq", bufs=2) as sqpool,
            tc.tile_pool(name="ln", bufs=4) as lpool,
            tc.tile_pool(name="prow", bufs=2, space="PSUM") as prow,
            tc.tile_pool(name="pmm", bufs=3, space="PSUM") as pmm,
            tc.tile_pool(name="dram", bufs=1, space="DRAM") as dpool,
        ):
            # ---- warmup AllReduce: very first instruction, no input
            # deps, so the ncfw wake + replica barrier start at t~0.
            with tc.high_priority():
                cc_w_in = dpool.tile([1, 1], f32)
                cc_w_out = dpool.tile([1, 1], f32, addr_space="Shared")
                nc.gpsimd.collective_compute(
                    "AllReduce",
                    ALU.add,
                    replica_groups=[list(range(N_CORES))],
                    ins=[cc_w_in.opt()],
                    outs=[cc_w_out.opt()],
                )

            ones_colb = cpool.tile([P, 1], bf16)  # lhsT for c2 row sums
            nc.vector.memset(ones_colb, 1.0)
            ones_col_f = cpool.tile([P, 1], f32)  # rhs for partition sum
            nc.vector.memset(ones_col_f, 1.0)
            ones_row_f = cpool.tile([1, P], f32)  # lhsT for broadcasts
            nc.vector.memset(ones_row_f, 1.0)

            # ---- loads (k-pair granularity so matmuls start early) ----
            ct = bpool.tile([P, KC, K], f8, name="cT")
            nc.sync.dma_start(ct[:, 0:2, :], cT_d[:, 0:2, :])
            nc.sync.dma_start(ct[:, 2:4, :], cT_d[:, 2:4, :])
            xt = bpool.tile([P, KC, BL], f8, name="xT")
            nc.sync.dma_start(xt[:, 0:2, :], xT_d[:, 0:2, :])
            nc.sync.dma_start(xt[:, 2:4, :], xT_d[:, 2:4, :])
            xrts = []
            for i in range(MT):
                xrt = xrp.tile([P, D], f8, tag="xr")
                nc.sync.dma_start(xrt, xr_d[i * P : (i + 1) * P, :])
                xrts.append(xrt)

            # ---- c2/2 replicated row (one-time) ----
            csqs = []
            for k in range(KC):
                csq = sqpool.tile([P, K], bf16, tag="csq")
                nc.vector.tensor_mul(csq, ct[:, k, :], ct[:, k, :])
                csqs.append(csq)
            c2h = cpool.tile([1, K], f32)  # c2 row
            for h in range(JH):
                rp = prow.tile([1, NJ], f32, tag="row")
                for k in range(KC):
                    nc.tensor.matmul(
                        rp,
                        lhsT=ones_colb,
                        rhs=csqs[k][:, h * NJ : (h + 1) * NJ],
                        start=(k == 0),
                        stop=(k == KC - 1),
                    )
                nc.scalar.activation(
                    c2h[0:1, h * NJ : (h + 1) * NJ], rp, AF.Copy, scale=0.5
                )
            c2rep = bpool.tile([P, K], f32, name="c2rep")
            pb = pmm.tile([P, K], f32, tag="mm")
            for h in range(JH):
                nc.tensor.matmul(
                    pb[:, h * NJ : (h + 1) * NJ],
                    lhsT=ones_row_f,
                    rhs=c2h[0:1, h * NJ : (h + 1) * NJ],
                    start=True,
                    stop=True,
                )
            nc.vector.tensor_copy(c2rep, pb)

            x2col = cpool.tile([P, MT], f32)
            sq_scr = cpool.tile([P, D], bf16)  # write-only scratch
            numbuf = bpool.tile([P, MT * K], f32)  # 64 KB/partition
            parts = cpool.tile([P, MT], f32)

            # ---- main pipeline ----
            # num = 1/(1+dist) = Sigmoid(-0.5*Ln(sq)): two ACT table
            # passes, no sqrt/add/reciprocal, and the Sigmoid pass's
            # accum_out produces the per-row partial sums for free.
            # DVE stages t = 2*psum - c2 into numbuf (frees PSUM), and
            # Ln reads it with scale=-1, bias=x2 -> ln(x2 - 2xc + c2).
            # Ln/Sigmoid run in batches of ACTB tiles so the ACT engine
            # amortizes its ~1.3us per-function table reloads.
            ACTB = 4
            for i in range(MT):
                ps = pmm.tile([P, K], f32, tag="mm")
                for h in range(JH):
                    psl = ps[:, h * NJ : (h + 1) * NJ]
                    for kp in range(KC // 2):
                        nc.tensor.matmul(
                            psl,
                            lhsT=xt[:, 2 * kp : 2 * kp + 2, i * P : (i + 1) * P],
                            rhs=ct[:, 2 * kp : 2 * kp + 2, h * NJ : (h + 1) * NJ],
                            start=(kp == 0),
                            stop=(kp == KC // 2 - 1),
                            perf_mode=DR,
                        )
                # x2 on DVE: square + row-reduce the row-major x tile
                nc.vector.tensor_mul(sq_scr, xrts[i], xrts[i])
                nc.vector.tensor_reduce(
                    x2col[:, i : i + 1], sq_scr, mybir.AxisListType.X, ALU.add
                )
                nsl = numbuf[:, i * K : (i + 1) * K]
                nc.vector.tensor_sub(nsl, ps, c2rep)  # xc - c2/2
                if i % ACTB == ACTB - 1:
                    lscs = {}
                    for j in range(i - ACTB + 1, i + 1):
                        lscs[j] = lpool.tile([P, K], f32, tag="ln", name=f"lsc{j}")
                        nc.scalar.activation(
                            lscs[j], numbuf[:, j * K : (j + 1) * K], AF.Ln,
                            bias=x2col[:, j : j + 1], scale=-2.0,
                        )
                    for j in range(i - ACTB + 1, i + 1):
                        nc.scalar.activation(
                            numbuf[:, j * K : (j + 1) * K], lscs[j], AF.Sigmoid,
                            scale=-0.5, accum_out=parts[:, j : j + 1],
                        )

            # ---- local total: [P,MT] -> [P,1] -> [1,1] ----
            pacc = cpool.tile([P, 1], f32)
            nc.vector.tensor_reduce(pacc, parts, mybir.AxisListType.X, ALU.add)
            sp = prow.tile([1, 1], f32, tag="row")
            nc.tensor.matmul(sp, lhsT=pacc, rhs=ones_col_f, start=True, stop=True)
            lsum = cpool.tile([1, 1], f32)
            nc.vector.tensor_copy(lsum, sp)

            # ---- AllReduce the scalar, then inv broadcast ----
            cc_in = dpool.tile([1, 1], f32)
            cc_out = dpool.tile([1, 1], f32, addr_space="Shared")
            nc.sync.dma_start(cc_in, lsum)
            nc.gpsimd.collective_compute(
                "AllReduce",
                ALU.add,
                replica_groups=[list(range(N_CORES))],
                ins=[cc_in.opt()],
                outs=[cc_out.opt()],
            )
            total = cpool.tile([1, 1], f32)
            nc.sync.dma_start(total, cc_out)
            inv = cpool.tile([1, 1], f32)
            nc.vector.reciprocal(inv, total)
            inv_ps = prow.tile([P, 1], f32, tag="row")
            nc.tensor.matmul(inv_ps, lhsT=ones_row_f, rhs=inv, start=True, stop=True)
            invb = cpool.tile([P, 1], f32)
            nc.vector.tensor_copy(invb, inv_ps)

            # ---- scale + store per 2 tiles, bf16 on the wire ----
            # ACT Copy(scale=invb) writes bf16 directly (no table
            # load); DVE chunks scale fp32 in-place then fast-cast.
            # Alternating engines halves the serial scale latency.
            obuf = bpool.tile([P, MT * K], bf16, name="obuf")
            NM = 2  # m-tiles per chunk
            for s in range(MT // NM):
                i0 = s * NM
                sl = numbuf[:, i0 * K : (i0 + NM) * K]
                osl = obuf[:, i0 * K : (i0 + NM) * K]
                if s % 2 == 0:
                    nc.scalar.activation(osl, sl, AF.Copy, scale=invb)
                else:
                    nc.vector.tensor_scalar_mul(sl, sl, invb)
                    nc.vector.tensor_copy(osl, sl)
                dst = out_d[i0 * P : (i0 + NM) * P, :].rearrange(
                    "(f p) c -> p f c", p=P
                )
                srcb = osl.rearrange("p (f c) -> p f c", f=NM)
                nc.sync.dma_start(dst, srcb)

    nc.finalize()
    return nc


def _get_bass():
    key = "nc"
    if key not in _CACHE:
        _CACHE[key] = _build_bass()
    return _CACHE[key]


def _host_prep(x: np.ndarray, clusters: np.ndarray):
    import ml_dtypes

    f8 = ml_dtypes.float8_e4m3
    # [128, KC, n] DoubleRow layout: tile[p, j, n] = src[n, j*128+p]
    cT = np.ascontiguousarray(
        clusters.T.reshape(KC, P, K).transpose(1, 0, 2)
    ).astype(f8)
    in_maps = []
    for c in range(N_CORES):
        xl = x[c * BL : (c + 1) * BL]
        xT_c = np.ascontiguousarray(
            xl.T.reshape(KC, P, BL).transpose(1, 0, 2)
        ).astype(f8)
        xr_c = np.ascontiguousarray(xl).astype(f8)
        in_maps.append({"xT": xT_c, "cT": cT, "xr": xr_c})
    return in_maps


def kernel(x: np.ndarray, clusters: np.ndarray) -> np.ndarray:
    from concourse.bass_utils import run_bass_kernel_spmd

    x = np.asarray(x, dtype=np.float32)
    clusters = np.asarray(clusters, dtype=np.float32)
    assert x.shape == (B, D) and clusters.shape == (K, D)

    in_maps = _host_prep(x, clusters)
    nc = _get_bass()
    res = run_bass_kernel_spmd(nc, in_maps, core_ids=list(range(N_CORES)))
    return np.concatenate(
        [np.asarray(r["out"]).astype(np.float32) for r in res.results], axis=0
    )


# revision 24
# speedup vs baseline: 3.4218x; 1.0075x over previous
"""Trainium2 Bass kernel for nn_ClusteringLayer (vq_codebook).

Computes, for x (B,D) and clusters (K,D):
    sq   = ||x_i||^2 - 2 x.clusters^T + ||c_j||^2     (B,K)
    dist = sqrt(sq)
    num  = 1 / (1 + dist)          (ALPHA=1 -> exponent -1)
    out  = num / sum(num)          (global scalar normalizer)

Sharding: data-parallel on batch across 8 NeuronCores; clusters
replicated; one 4-byte AllReduce for the normalizer.

Host-side prep is layout/precision only: x and clusters are shipped as
fp8e4m3 in a DoubleRow-friendly layout ([128, 4 k-subtiles, n]) plus a
row-major fp8 copy of x for the on-device row-norm computation. Using
the SAME fp8 values for the GEMM and for x2/c2 makes dist the exact
distance between the quantized points (errors correlate and largely
cancel), ~0.1% relative error on num vs the 2e-2 gate.

Per-core device program (Bl = B/8 = 2048 local rows, 16 m-tiles):
  - warmup 4-byte AllReduce as the very first instruction (starts the
    ncfw wake/barrier immediately; result unused)
  - c2/2 row via Square + ones-matmuls, replicated across partitions
    with one K=1 fp32 matmul -> c2rep (128,1024)
  - x2 per m-tile via ACT Square with accum_out on row-major x tiles
  - per m-tile: 4 fp8 DoubleRow matmuls -> psum = x.c^T;
    GpSimd psum -= c2rep; ACT dist = Sqrt(-2*psum + x2[P,1]);
    GpSimd dist += 1; DVE reciprocal_approx_fast -> num;
    DVE tensor_reduce -> per-tile partial sums
  - cross-partition sum via one fp32 matmul; AllReduce scalar;
    reciprocal -> inv broadcast via K=1 fp32 matmul
  - scale (DVE/GpSimd alternating per 2-tile chunk) + immediate DMA out
"""

import numpy as np

B, D, K = 16384, 512, 1024
N_CORES = 8
BL = B // N_CORES        # 2048 rows per core
P = 128                  # partitions
MT = BL // P             # 16 m-tiles per core
KC = D // P              # 4 contraction subtiles
NJ = 512                 # matmul moving free dim limit (1 PSUM bank)
JH = K // NJ             # 2 j-halves

OUT_BF16 = True          # ship output as bf16, upcast on host

_CACHE = {}


def _build_bass():
    import concourse.bass as bass  # noqa: F401
    import concourse.mybir as mybir
    import concourse.tile as tile
    from concourse import bacc

    f32 = mybir.dt.float32
    bf16 = mybir.dt.bfloat16
    f8 = mybir.dt.float8e4
    AF = mybir.ActivationFunctionType
    ALU = mybir.AluOpType
    DR = mybir.MatmulPerfMode.DoubleRow
    out_dt = bf16 if OUT_BF16 else f32

    nc = bacc.Bacc(
        "TRN2", target_bir_lowering=False, debug=False, num_devices=N_CORES
    )
    xT_d = nc.dram_tensor("xT", [P, KC, BL], f8, kind="ExternalInput").ap()
    cT_d = nc.dram_tensor("cT", [P, KC, K], f8, kind="ExternalInput").ap()
    xr_d = nc.dram_tensor("xr", [BL, D], f8, kind="ExternalInput").ap()
    out_d = nc.dram_tensor("out", [BL, K], out_dt, kind="ExternalOutput").ap()

    with tile.TileContext(nc) as tc:
        with (
            tc.tile_pool(name="const", bufs=1) as cpool,
            tc.tile_pool(name="big", bufs=1) as bpool,
            tc.tile_pool(name="xrp", bufs=4) as xrp,
            tc.tile_pool(name="sq", bufs=2) as sqpool,
            tc.tile_pool(name="ln", bufs=8) as lpool,
            tc.tile_pool(name="prow", bufs=2, space="PSUM") as prow,
            tc.tile_pool(name="pmm", bufs=3, space="PSUM") as pmm,
            tc.tile_pool(name="dram", bufs=1, space="DRAM") as dpool,
        ):
            # (No warmup AllReduce: the replica barrier alone performs
            # the ncfw wake, and a warmup op would occupy the first
            # post-barrier service slot, delaying the real AllReduce.)
            ones_colb = cpool.tile([P, 1], bf16)  # lhsT for c2 row sums
            nc.vector.memset(ones_colb, 1.0)
            ones_col_f = cpool.tile([P, 1], f32)  # rhs for partition sum
            nc.vector.memset(ones_col_f, 1.0)
            ones_row_f = cpool.tile([1, P], f32)  # lhsT for broadcasts
            nc.vector.memset(ones_row_f, 1.0)

            # ---- loads (k-pair granularity so matmuls start early) ----
            ct = bpool.tile([P, KC, K], f8, name="cT")
            nc.sync.dma_start(ct[:, 0:2, :], cT_d[:, 0:2, :])
            nc.sync.dma_start(ct[:, 2:4, :], cT_d[:, 2:4, :])
            xt = bpool.tile([P, KC, BL], f8, name="xT")
            nc.sync.dma_start(xt[:, 0:2, :], xT_d[:, 0:2, :])
            nc.sync.dma_start(xt[:, 2:4, :], xT_d[:, 2:4, :])
            xrts = []
            for i in range(MT):
                xrt = xrp.tile([P, D], f8, tag="xr")
                nc.sync.dma_start(xrt, xr_d[i * P : (i + 1) * P, :])
                xrts.append(xrt)

            # ---- c2/2 replicated row (one-time) ----
            csqs = []
            for k in range(KC):
                csq = sqpool.tile([P, K], bf16, tag="csq")
                nc.vector.tensor_mul(csq, ct[:, k, :], ct[:, k, :])
                csqs.append(csq)
            c2h = cpool.tile([1, K], f32)  # c2 row
            for h in range(JH):
                rp = prow.tile([1, NJ], f32, tag="row")
                for k in range(KC):
                    nc.tensor.matmul(
                        rp,
                        lhsT=ones_colb,
                        rhs=csqs[k][:, h * NJ : (h + 1) * NJ],
                        start=(k == 0),
                        stop=(k == KC - 1),
                    )
                nc.scalar.activation(
                    c2h[0:1, h * NJ : (h + 1) * NJ], rp, AF.Copy, scale=0.5
                )
            c2rep = bpool.tile([P, K], f32, name="c2rep")
            pb = pmm.tile([P, K], f32, tag="mm")
            for h in range(JH):
                nc.tensor.matmul(
                    pb[:, h * NJ : (h + 1) * NJ],
                    lhsT=ones_row_f,
                    rhs=c2h[0:1, h * NJ : (h + 1) * NJ],
                    start=True,
                    stop=True,
                )
            nc.vector.tensor_copy(c2rep, pb)

            x2col = cpool.tile([P, MT], f32)
            sq_scr = cpool.tile([P, D], bf16)  # write-only scratch
            numbuf = bpool.tile([P, MT * K], f32)  # 64 KB/partition
            parts = cpool.tile([P, MT], f32)

            # ---- main pipeline ----
            # num = 1/(1+dist) = Sigmoid(-0.5*Ln(sq)): two ACT table
            # passes, no sqrt/add/reciprocal, and the Sigmoid pass's
            # accum_out produces the per-row partial sums for free.
            # DVE stages t = 2*psum - c2 into numbuf (frees PSUM), and
            # Ln reads it with scale=-1, bias=x2 -> ln(x2 - 2xc + c2).
            # Ln/Sigmoid run in batches so the ACT engine amortizes its
            # ~1.5us per-function table reloads; the batches shrink
            # toward the end so the last tiles' sum arrives sooner.
            act_batch_end = {7: 8, 13: 6, 15: 2}
            for i in range(MT):
                ps = pmm.tile([P, K], f32, tag="mm")
                for h in range(JH):
                    psl = ps[:, h * NJ : (h + 1) * NJ]
                    for kp in range(KC // 2):
                        nc.tensor.matmul(
                            psl,
                            lhsT=xt[:, 2 * kp : 2 * kp + 2, i * P : (i + 1) * P],
                            rhs=ct[:, 2 * kp : 2 * kp + 2, h * NJ : (h + 1) * NJ],
                            start=(kp == 0),
                            stop=(kp == KC // 2 - 1),
                            perf_mode=DR,
                        )
                # x2 on DVE: square + row-reduce the row-major x tile
                nc.vector.tensor_mul(sq_scr, xrts[i], xrts[i])
                nc.vector.tensor_reduce(
                    x2col[:, i : i + 1], sq_scr, mybir.AxisListType.X, ALU.add
                )
                nsl = numbuf[:, i * K : (i + 1) * K]
                nc.vector.tensor_sub(nsl, ps, c2rep)  # xc - c2/2
                if i in act_batch_end:
                    ACTB = act_batch_end[i]
                    lscs = {}
                    for j in range(i - ACTB + 1, i + 1):
                        lscs[j] = lpool.tile([P, K], f32, tag="ln", name=f"lsc{j}")
                        nc.scalar.activation(
                            lscs[j], numbuf[:, j * K : (j + 1) * K], AF.Ln,
                            bias=x2col[:, j : j + 1], scale=-2.0,
                        )
                    for j in range(i - ACTB + 1, i + 1):
                        nc.scalar.activation(
                            numbuf[:, j * K : (j + 1) * K], lscs[j], AF.Sigmoid,
                            scale=-0.5, accum_out=parts[:, j : j + 1],
                        )

            # ---- local total: [P,MT] -> [P,1] -> [1,1] ----
            pacc = cpool.tile([P, 1], f32)
            nc.vector.tensor_reduce(pacc, parts, mybir.AxisListType.X, ALU.add)
            sp = prow.tile([1, 1], f32, tag="row")
            nc.tensor.matmul(sp, lhsT=pacc, rhs=ones_col_f, start=True, stop=True)
            lsum = cpool.tile([1, 1], f32)
            nc.vector.tensor_copy(lsum, sp)

            # ---- AllReduce the scalar, then inv broadcast ----
            cc_in = dpool.tile([1, 1], f32)
            cc_out = dpool.tile([1, 1], f32, addr_space="Shared")
            nc.sync.dma_start(cc_in, lsum)
            nc.gpsimd.collective_compute(
                "AllReduce",
                ALU.add,
                replica_groups=[list(range(N_CORES))],
                ins=[cc_in.opt()],
                outs=[cc_out.opt()],
            )
            total = cpool.tile([1, 1], f32)
            nc.sync.dma_start(total, cc_out)
            inv = cpool.tile([1, 1], f32)
            nc.vector.reciprocal(inv, total)
            inv_ps = prow.tile([P, 1], f32, tag="row")
            nc.tensor.matmul(inv_ps, lhsT=ones_row_f, rhs=inv, start=True, stop=True)
            invb = cpool.tile([P, 1], f32)
            nc.vector.tensor_copy(invb, inv_ps)

            # ---- scale + store per 2 tiles, bf16 on the wire ----
            # ACT Copy(scale=invb) writes bf16 directly (no table
            # load); DVE chunks scale fp32 in-place then fast-cast.
            # Alternating engines halves the serial scale latency.
            obuf = bpool.tile([P, MT * K], bf16, name="obuf")
            NM = 1  # m-tiles per chunk
            for s in range(MT // NM):
                i0 = s * NM
                sl = numbuf[:, i0 * K : (i0 + NM) * K]
                osl = obuf[:, i0 * K : (i0 + NM) * K]
                if s % 2 == 0:
                    nc.scalar.activation(osl, sl, AF.Copy, scale=invb)
                else:
                    nc.vector.tensor_scalar_mul(sl, sl, invb)
                    nc.vector.tensor_copy(osl, sl)
                dst = out_d[i0 * P : (i0 + NM) * P, :].rearrange(
                    "(f p) c -> p f c", p=P
                )
                srcb = osl.rearrange("p (f c) -> p f c", f=NM)
                nc.sync.dma_start(dst, srcb)

    nc.finalize()
    return nc


def _get_bass():
    key = "nc"
    if key not in _CACHE:
        _CACHE[key] = _build_bass()
    return _CACHE[key]


def _host_prep(x: np.ndarray, clusters: np.ndarray):
    import ml_dtypes

    f8 = ml_dtypes.float8_e4m3
    # [128, KC, n] DoubleRow layout: tile[p, j, n] = src[n, j*128+p]
    cT = np.ascontiguousarray(
        clusters.T.reshape(KC, P, K).transpose(1, 0, 2)
    ).astype(f8)
    in_maps = []
    for c in range(N_CORES):
        xl = x[c * BL : (c + 1) * BL]
        xT_c = np.ascontiguousarray(
            xl.T.reshape(KC, P, BL).transpose(1, 0, 2)
        ).astype(f8)
        xr_c = np.ascontiguousarray(xl).astype(f8)
        in_maps.append({"xT": xT_c, "cT": cT, "xr": xr_c})
    return in_maps


def kernel(x: np.ndarray, clusters: np.ndarray) -> np.ndarray:
    from concourse.bass_utils import run_bass_kernel_spmd

    x = np.asarray(x, dtype=np.float32)
    clusters = np.asarray(clusters, dtype=np.float32)
    assert x.shape == (B, D) and clusters.shape == (K, D)

    in_maps = _host_prep(x, clusters)
    nc = _get_bass()
    res = run_bass_kernel_spmd(nc, in_maps, core_ids=list(range(N_CORES)))
    return np.concatenate(
        [np.asarray(r["out"]).astype(np.float32) for r in res.results], axis=0
    )


# revision 25
# speedup vs baseline: 3.4397x; 1.0052x over previous
"""Trainium2 Bass kernel for nn_ClusteringLayer (vq_codebook).

Computes, for x (B,D) and clusters (K,D):
    sq   = ||x_i||^2 - 2 x.clusters^T + ||c_j||^2     (B,K)
    dist = sqrt(sq)
    num  = 1 / (1 + dist)          (ALPHA=1 -> exponent -1)
    out  = num / sum(num)          (global scalar normalizer)

Sharding: data-parallel on batch across 8 NeuronCores; clusters
replicated; one 4-byte AllReduce for the normalizer.

Host-side prep is layout/precision only: x and clusters are shipped as
fp8e4m3 in a DoubleRow-friendly layout ([128, 4 k-subtiles, n]) plus a
row-major fp8 copy of x for the on-device row-norm computation. Using
the SAME fp8 values for the GEMM and for x2/c2 makes dist the exact
distance between the quantized points (errors correlate and largely
cancel), ~0.1% relative error on num vs the 2e-2 gate.

Per-core device program (Bl = B/8 = 2048 local rows, 16 m-tiles):
  - warmup 4-byte AllReduce as the very first instruction (starts the
    ncfw wake/barrier immediately; result unused)
  - c2/2 row via Square + ones-matmuls, replicated across partitions
    with one K=1 fp32 matmul -> c2rep (128,1024)
  - x2 per m-tile via ACT Square with accum_out on row-major x tiles
  - per m-tile: 4 fp8 DoubleRow matmuls -> psum = x.c^T;
    GpSimd psum -= c2rep; ACT dist = Sqrt(-2*psum + x2[P,1]);
    GpSimd dist += 1; DVE reciprocal_approx_fast -> num;
    DVE tensor_reduce -> per-tile partial sums
  - cross-partition sum via one fp32 matmul; AllReduce scalar;
    reciprocal -> inv broadcast via K=1 fp32 matmul
  - scale (DVE/GpSimd alternating per 2-tile chunk) + immediate DMA out
"""

import numpy as np

B, D, K = 16384, 512, 1024
N_CORES = 8
BL = B // N_CORES        # 2048 rows per core
P = 128                  # partitions
MT = BL // P             # 16 m-tiles per core
KC = D // P              # 4 contraction subtiles
NJ = 512                 # matmul moving free dim limit (1 PSUM bank)
JH = K // NJ             # 2 j-halves

OUT_BF16 = True          # ship output as bf16, upcast on host

_CACHE = {}


def _build_bass():
    import concourse.bass as bass  # noqa: F401
    import concourse.mybir as mybir
    import concourse.tile as tile
    from concourse import bacc

    f32 = mybir.dt.float32
    bf16 = mybir.dt.bfloat16
    f8 = mybir.dt.float8e4
    AF = mybir.ActivationFunctionType
    ALU = mybir.AluOpType
    DR = mybir.MatmulPerfMode.DoubleRow
    out_dt = bf16 if OUT_BF16 else f32

    nc = bacc.Bacc(
        "TRN2", target_bir_lowering=False, debug=False, num_devices=N_CORES
    )
    xT_d = nc.dram_tensor("xT", [P, KC, BL], f8, kind="ExternalInput").ap()
    cT_d = nc.dram_tensor("cT", [P, KC, K], f8, kind="ExternalInput").ap()
    xr_d = nc.dram_tensor("xr", [BL, D], f8, kind="ExternalInput").ap()
    out_d = nc.dram_tensor("out", [BL, K], out_dt, kind="ExternalOutput").ap()

    with tile.TileContext(nc) as tc:
        with (
            tc.tile_pool(name="const", bufs=1) as cpool,
            tc.tile_pool(name="big", bufs=1) as bpool,
            tc.tile_pool(name="xrp", bufs=4) as xrp,
            tc.tile_pool(name="sq", bufs=2) as sqpool,
            tc.tile_pool(name="ln", bufs=8) as lpool,
            tc.tile_pool(name="prow", bufs=2, space="PSUM") as prow,
            tc.tile_pool(name="pmm", bufs=3, space="PSUM") as pmm,
            tc.tile_pool(name="dram", bufs=1, space="DRAM") as dpool,
        ):
            # (No warmup AllReduce: the replica barrier alone performs
            # the ncfw wake, and a warmup op would occupy the first
            # post-barrier service slot, delaying the real AllReduce.)
            ones_colb = cpool.tile([P, 1], bf16)  # lhsT for c2 row sums
            nc.vector.memset(ones_colb, 1.0)
            ones_col_f = cpool.tile([P, 1], f32)  # rhs for partition sum
            nc.vector.memset(ones_col_f, 1.0)
            ones_row_f = cpool.tile([1, P], f32)  # lhsT for broadcasts
            nc.vector.memset(ones_row_f, 1.0)

            # ---- loads (k-pair granularity so matmuls start early) ----
            ct = bpool.tile([P, KC, K], f8, name="cT")
            nc.sync.dma_start(ct[:, 0:2, :], cT_d[:, 0:2, :])
            nc.sync.dma_start(ct[:, 2:4, :], cT_d[:, 2:4, :])
            xt = bpool.tile([P, KC, BL], f8, name="xT")
            nc.sync.dma_start(xt[:, 0:2, :], xT_d[:, 0:2, :])
            nc.sync.dma_start(xt[:, 2:4, :], xT_d[:, 2:4, :])
            xrts = []
            for i in range(MT):
                xrt = xrp.tile([P, D], f8, tag="xr")
                nc.sync.dma_start(xrt, xr_d[i * P : (i + 1) * P, :])
                xrts.append(xrt)

            # ---- c2/2 replicated row (one-time) ----
            csqs = []
            for k in range(KC):
                csq = sqpool.tile([P, K], bf16, tag="csq")
                nc.vector.tensor_mul(csq, ct[:, k, :], ct[:, k, :])
                csqs.append(csq)
            c2h = cpool.tile([1, K], f32)  # c2 row
            for h in range(JH):
                rp = prow.tile([1, NJ], f32, tag="row")
                for k in range(KC):
                    nc.tensor.matmul(
                        rp,
                        lhsT=ones_colb,
                        rhs=csqs[k][:, h * NJ : (h + 1) * NJ],
                        start=(k == 0),
                        stop=(k == KC - 1),
                    )
                nc.scalar.activation(
                    c2h[0:1, h * NJ : (h + 1) * NJ], rp, AF.Copy, scale=0.5
                )
            c2rep = bpool.tile([P, K], f32, name="c2rep")
            pb = pmm.tile([P, K], f32, tag="mm")
            for h in range(JH):
                nc.tensor.matmul(
                    pb[:, h * NJ : (h + 1) * NJ],
                    lhsT=ones_row_f,
                    rhs=c2h[0:1, h * NJ : (h + 1) * NJ],
                    start=True,
                    stop=True,
                )
            nc.vector.tensor_copy(c2rep, pb)

            x2col = cpool.tile([P, MT], f32)
            sq_scr = cpool.tile([P, D], bf16)  # write-only scratch
            numbuf = bpool.tile([P, MT * K], f32)  # 64 KB/partition
            parts = cpool.tile([P, MT], f32)

            # ---- main pipeline ----
            # num = 1/(1+dist) = Sigmoid(-0.5*Ln(sq)): two ACT table
            # passes, no sqrt/add/reciprocal, and the Sigmoid pass's
            # accum_out produces the per-row partial sums for free.
            # DVE stages t = 2*psum - c2 into numbuf (frees PSUM), and
            # Ln reads it with scale=-1, bias=x2 -> ln(x2 - 2xc + c2).
            # Ln/Sigmoid run in batches so the ACT engine amortizes its
            # ~1.5us per-function table reloads; the batches shrink
            # toward the end so the last tiles' sum arrives sooner.
            act_batch_end = {3: 4, 9: 6, 13: 4, 15: 2}
            for i in range(MT):
                ps = pmm.tile([P, K], f32, tag="mm")
                for h in range(JH):
                    psl = ps[:, h * NJ : (h + 1) * NJ]
                    for kp in range(KC // 2):
                        nc.tensor.matmul(
                            psl,
                            lhsT=xt[:, 2 * kp : 2 * kp + 2, i * P : (i + 1) * P],
                            rhs=ct[:, 2 * kp : 2 * kp + 2, h * NJ : (h + 1) * NJ],
                            start=(kp == 0),
                            stop=(kp == KC // 2 - 1),
                            perf_mode=DR,
                        )
                # x2 on DVE: square + row-reduce the row-major x tile
                nc.vector.tensor_mul(sq_scr, xrts[i], xrts[i])
                nc.vector.tensor_reduce(
                    x2col[:, i : i + 1], sq_scr, mybir.AxisListType.X, ALU.add
                )
                nsl = numbuf[:, i * K : (i + 1) * K]
                nc.vector.tensor_sub(nsl, ps, c2rep)  # xc - c2/2
                if i in act_batch_end:
                    ACTB = act_batch_end[i]
                    lscs = {}
                    for j in range(i - ACTB + 1, i + 1):
                        lscs[j] = lpool.tile([P, K], f32, tag="ln", name=f"lsc{j}")
                        nc.scalar.activation(
                            lscs[j], numbuf[:, j * K : (j + 1) * K], AF.Ln,
                            bias=x2col[:, j : j + 1], scale=-2.0,
                        )
                    for j in range(i - ACTB + 1, i + 1):
                        nc.scalar.activation(
                            numbuf[:, j * K : (j + 1) * K], lscs[j], AF.Sigmoid,
                            scale=-0.5, accum_out=parts[:, j : j + 1],
                        )

            # ---- local total: [P,MT] -> [P,1] -> [1,1] ----
            pacc = cpool.tile([P, 1], f32)
            nc.vector.tensor_reduce(pacc, parts, mybir.AxisListType.X, ALU.add)
            sp = prow.tile([1, 1], f32, tag="row")
            nc.tensor.matmul(sp, lhsT=pacc, rhs=ones_col_f, start=True, stop=True)
            lsum = cpool.tile([1, 1], f32)
            nc.vector.tensor_copy(lsum, sp)

            # ---- AllReduce the scalar, then inv broadcast ----
            cc_in = dpool.tile([1, 1], f32)
            cc_out = dpool.tile([1, 1], f32, addr_space="Shared")
            nc.sync.dma_start(cc_in, lsum)
            nc.gpsimd.collective_compute(
                "AllReduce",
                ALU.add,
                replica_groups=[list(range(N_CORES))],
                ins=[cc_in.opt()],
                outs=[cc_out.opt()],
            )
            total = cpool.tile([1, 1], f32)
            nc.sync.dma_start(total, cc_out)
            inv = cpool.tile([1, 1], f32)
            nc.vector.reciprocal(inv, total)
            inv_ps = prow.tile([P, 1], f32, tag="row")
            nc.tensor.matmul(inv_ps, lhsT=ones_row_f, rhs=inv, start=True, stop=True)
            invb = cpool.tile([P, 1], f32)
            nc.vector.tensor_copy(invb, inv_ps)

            # ---- scale + store per 2 tiles, bf16 on the wire ----
            # ACT Copy(scale=invb) writes bf16 directly (no table
            # load); DVE chunks scale fp32 in-place then fast-cast.
            # Alternating engines halves the serial scale latency.
            obuf = bpool.tile([P, MT * K], bf16, name="obuf")
            NM = 1  # m-tiles per chunk
            for s in range(MT // NM):
                i0 = s * NM
                sl = numbuf[:, i0 * K : (i0 + NM) * K]
                osl = obuf[:, i0 * K : (i0 + NM) * K]
                if s % 2 == 0:
                    nc.scalar.activation(osl, sl, AF.Copy, scale=invb)
                else:
                    nc.vector.tensor_scalar_mul(sl, sl, invb)
                    nc.vector.tensor_copy(osl, sl)
                dst = out_d[i0 * P : (i0 + NM) * P, :].rearrange(
                    "(f p) c -> p f c", p=P
                )
                srcb = osl.rearrange("p (f c) -> p f c", f=NM)
                nc.sync.dma_start(dst, srcb)

    nc.finalize()
    return nc


def _get_bass():
    key = "nc"
    if key not in _CACHE:
        _CACHE[key] = _build_bass()
    return _CACHE[key]


def _host_prep(x: np.ndarray, clusters: np.ndarray):
    import ml_dtypes

    f8 = ml_dtypes.float8_e4m3
    # [128, KC, n] DoubleRow layout: tile[p, j, n] = src[n, j*128+p]
    cT = np.ascontiguousarray(
        clusters.T.reshape(KC, P, K).transpose(1, 0, 2)
    ).astype(f8)
    in_maps = []
    for c in range(N_CORES):
        xl = x[c * BL : (c + 1) * BL]
        xT_c = np.ascontiguousarray(
            xl.T.reshape(KC, P, BL).transpose(1, 0, 2)
        ).astype(f8)
        xr_c = np.ascontiguousarray(xl).astype(f8)
        in_maps.append({"xT": xT_c, "cT": cT, "xr": xr_c})
    return in_maps


def kernel(x: np.ndarray, clusters: np.ndarray) -> np.ndarray:
    from concourse.bass_utils import run_bass_kernel_spmd

    x = np.asarray(x, dtype=np.float32)
    clusters = np.asarray(clusters, dtype=np.float32)
    assert x.shape == (B, D) and clusters.shape == (K, D)

    in_maps = _host_prep(x, clusters)
    nc = _get_bass()
    res = run_bass_kernel_spmd(nc, in_maps, core_ids=list(range(N_CORES)))
    return np.concatenate(
        [np.asarray(r["out"]).astype(np.float32) for r in res.results], axis=0
    )
